# revision 21
# baseline (speedup 1.0000x reference)
"""Causal multi-head attention (B=2, T=2048, D=1024, H=16) on 8 TRN2 NeuronCores.

Sharding: core c handles batch b = c//4 and head-group hg = c%4 (4 heads each).
Per core, everything is computed in transposed-friendly layouts so no on-chip
transposes are needed:

  - qT, kT      [128=2 heads x 64, T]   (pair-stacked projections, bf16)
  - v'          [T-tiles, 4 heads x 65] (natural layout, 65th col = 1.0 so the
                                         PV matmul also produces the softmax
                                         denominator as psum row 64)
  - scores^T    [s=128, q<=512]  per s-tile, causal-restricted column range
  - softmax     exp on ACT (no max subtraction: |scores| is O(1) for this
                 problem), denominator via the ones-column, reciprocal on DVE,
                 broadcast across partitions with a K=1 PE matmul
  - out-proj    per-head K=64 matmuls accumulating all 4 heads into one PSUM
                tile; the final bias bo and the 4-way head-group reduction
                happen on the host (that reduction is the TP-unshard step).

Outputs per core: partial out [T, D] f32, kT [256, T] f32, v [T, 256] f32.
Host sums the 4 partials per batch, adds bo, and reassembles k/v to
[B, H, T, 64]. Returns (out, (k, v)) matching the reference pytree.
"""

import numpy as np
import ml_dtypes

B = 2
T = 2048
D = 1024
H = 16
HD = 64
HPC = 4            # heads per core
DHP = HPC * HD     # 256
VW = HPC * (HD + 1)  # 260, v' width incl. ones columns
CH = 512           # q-chunk width
BF = ml_dtypes.bfloat16

_CACHE = {}


def _build_nc(t, d, dbg=False):
    import concourse.bass as bass
    import concourse.tile as tile
    from concourse import bacc, mybir

    bf16 = mybir.dt.bfloat16
    f32 = mybir.dt.float32
    Exp = mybir.ActivationFunctionType.Exp

    ct = d // 128      # contraction tiles
    nt = t // 128      # 128-row t-tiles
    nch = t // CH      # q-chunks
    spc = CH // 128    # s-tiles per chunk (4)

    nc = bacc.Bacc()

    def act_recip(out_ap, in_ap):
        eng = nc.scalar
        ins = [eng.lower_ap(in_ap)]
        for val in (0.0, 1.0, 0.0):  # bias, scale, alpha
            ins.append(mybir.ImmediateValue(dtype=f32, value=val))
        return eng.add_instruction(mybir.InstActivation(
            name=nc.get_next_instruction_name(),
            func=mybir.ActivationFunctionType.Reciprocal,
            ins=ins, outs=[eng.lower_ap(out_ap)]))
    xT = nc.declare_dram_parameter("xT", [d, t], bf16, isOutput=False)
    wqT = nc.declare_dram_parameter("wqT", [d, DHP], bf16, isOutput=False)
    wkT = nc.declare_dram_parameter("wkT", [d, DHP], bf16, isOutput=False)
    wvTa = nc.declare_dram_parameter("wvTa", [d, VW], bf16, isOutput=False)
    woT = nc.declare_dram_parameter("woT", [128, 2, d], bf16, isOutput=False)
    bqkc = nc.declare_dram_parameter("bqkc", [128, 4], f32, isOutput=False)
    bvar = nc.declare_dram_parameter("bvar", [1, VW], f32, isOutput=False)
    onesP = nc.declare_dram_parameter("onesP", [128, 512], bf16, isOutput=False)
    tri = nc.declare_dram_parameter("tri", [128, 128], bf16, isOutput=False)

    outp = nc.declare_dram_parameter("outp", [t, d], f32, isOutput=True)
    kout = nc.declare_dram_parameter("kout", [DHP, t], f32, isOutput=True)
    vout = nc.declare_dram_parameter("vout", [t, DHP], f32, isOutput=True)
    if dbg:
        dbg_e = nc.dram_tensor("dbg_e", [CH // 128, 128, CH], f32,
                               kind="ExternalOutput")
        dbg_cx = nc.dram_tensor("dbg_cx", [65, CH], f32, kind="ExternalOutput")
        dbg_rb = nc.dram_tensor("dbg_rb", [64, CH], f32, kind="ExternalOutput")
        dbg_cn = nc.dram_tensor("dbg_cn", [64, CH], f32, kind="ExternalOutput")
        dbg_cnh = nc.dram_tensor("dbg_cnh", [HPC, 64, CH], f32,
                                 kind="ExternalOutput")
        dbg_op = nc.dram_tensor("dbg_op", [128, min(512, d)], f32,
                                kind="ExternalOutput")

    with tile.TileContext(nc) as tc, \
         nc.allow_low_precision(reason="bf16 compute pipeline"):
        with tc.tile_pool(name="persist", bufs=1) as pp:
            xt = pp.tile([128, ct, t], bf16)
            wq = pp.tile([128, ct, DHP], bf16)
            wk = pp.tile([128, ct, DHP], bf16)
            wv = pp.tile([128, ct, VW], bf16)
            wo = pp.tile([128, 2, d], bf16)
            bqk = pp.tile([128, 4], f32)
            ones = pp.tile([128, 512], bf16)
            trim = pp.tile([128, 128], bf16)
            qT = [pp.tile([128, t], bf16, name=f"qT{p}", tag=f"qT{p}") for p in range(2)]
            kT = [pp.tile([128, t], bf16, name=f"kT{p}", tag=f"kT{p}") for p in range(2)]
            vv = pp.tile([128, nt, VW], bf16)

            xTr = xT.rearrange("(a p) t -> p a t", p=128)
            nc.sync.dma_start(out=wq[:], in_=wqT.rearrange("(a p) m -> p a m", p=128))
            nc.sync.dma_start(out=wk[:], in_=wkT.rearrange("(a p) m -> p a m", p=128))
            for a in range(ct):
                nc.sync.dma_start(out=xt[:, a, :], in_=xTr[:, a, :])
            nc.sync.dma_start(out=wv[:], in_=wvTa.rearrange("(a p) m -> p a m", p=128))
            nc.sync.dma_start(out=wo[:], in_=woT[:])
            nc.sync.dma_start(out=bqk[:], in_=bqkc[:])
            # bias broadcast across partitions for the v evacuation
            bva_bc = pp.tile([128, VW], f32)
            _bap = bvar[:, :]
            nc.sync.dma_start(
                out=bva_bc[:],
                in_=bass.AP(tensor=_bap.tensor, offset=_bap.offset,
                            ap=[[0, 128]] + list(_bap.ap[1:])))
            nc.sync.dma_start(out=ones[:], in_=onesP[:])
            nc.sync.dma_start(out=trim[:], in_=tri[:])

            # ---- projections: qT/kT pair-stacked [128, t], v' natural ----
            with tc.tile_pool(name="pj_ps", bufs=3, space="PSUM") as pjp, \
                 tc.tile_pool(name="pj_sb", bufs=3) as pjs:
                for pair in range(2):
                    for tj in range(t // 512):
                        ts = slice(tj * 512, tj * 512 + 512)
                        ms = slice(pair * 128, pair * 128 + 128)
                        for bi, (name, w, dst) in enumerate(
                            (("q", wq, qT), ("k", wk, kT)),
                        ):
                            ps = pjp.tile([128, 512], f32, tag="pj")
                            for a in range(ct):
                                nc.tensor.matmul(
                                    ps[:], w[:, a, ms], xt[:, a, ts],
                                    start=(a == 0), stop=(a == ct - 1))
                            nc.vector.tensor_scalar_add(
                                dst[pair][:, ts], ps[:],
                                bqk[:, 2 * bi + pair: 2 * bi + pair + 1])
                            if name == "k":
                                kf = pjs.tile([128, 512], f32, tag="kf")
                                nc.vector.tensor_scalar_add(
                                    kf[:], ps[:],
                                    bqk[:, 2 * bi + pair: 2 * bi + pair + 1])
                                nc.gpsimd.dma_start(
                                    out=kout[ms, ts], in_=kf[:])
                for tj in range(nt):
                    ts = slice(tj * 128, tj * 128 + 128)
                    ps = pjp.tile([128, VW], f32, tag="pjv")
                    for a in range(ct):
                        nc.tensor.matmul(
                            ps[:], xt[:, a, ts], wv[:, a, :],
                            start=(a == 0), stop=(a == ct - 1))
                    nc.vector.scalar_tensor_tensor(
                        out=vv[:, tj, :], in0=ps[:], scalar=1.0,
                        in1=bva_bc[:], op0=mybir.AluOpType.mult,
                        op1=mybir.AluOpType.add)
                    vf = pjs.tile([128, DHP], f32, tag="vf")
                    pv4 = ps.rearrange("p (h c) -> p h c", h=HPC)
                    bv4 = bva_bc.rearrange("p (h c) -> p h c", h=HPC)
                    nc.vector.scalar_tensor_tensor(
                        out=vf.rearrange("p (h c) -> p h c", h=HPC),
                        in0=pv4[:, :, 0:HD], scalar=1.0,
                        in1=bv4[:, :, 0:HD], op0=mybir.AluOpType.mult,
                        op1=mybir.AluOpType.add)
                    nc.gpsimd.dma_start(out=vout[ts, :], in_=vf[:])

            # ---- attention + output projection, per q-chunk ----
            with tc.tile_pool(name="st_ps", bufs=3, space="PSUM") as stp, \
                 tc.tile_pool(name="cx_ps", bufs=2, space="PSUM") as cxp, \
                 tc.tile_pool(name="rb_ps", bufs=1, space="PSUM") as rbp, \
                 tc.tile_pool(name="op_ps", bufs=2, space="PSUM") as opp, \
                 tc.tile_pool(name="at_sb", bufs=4) as ats, \
                 tc.tile_pool(name="cx_sb", bufs=8) as cxs, \
                 tc.tile_pool(name="ot_sb", bufs=4) as ots:
                for cj in range(nch):
                    q0 = cj * CH
                    cnp = [cxs.tile([128, CH], bf16, name=f"cnp{cj}_{p}",
                                    tag=f"cnp{p}") for p in range(2)]
                    for h in range(HPC):
                        pair, po = h // 2, (h % 2) * 64
                        qTh = qT[pair][po:po + 64, :]
                        kTh = kT[pair][po:po + 64, :]
                        nst = spc * (cj + 1)
                        cx = cxp.tile([65, CH], f32, tag="cx")
                        for i in range(nst):
                            rel = i - spc * cj
                            qoff = max(rel, 0) * 128
                            w = CH - qoff
                            st = stp.tile([128, CH], f32, tag="st")
                            nc.tensor.matmul(
                                st[:, 0:w],
                                kTh[:, i * 128:(i + 1) * 128],
                                qTh[:, q0 + qoff:q0 + CH],
                                start=True, stop=True)
                            e = ats.tile([128, CH], bf16, tag="e")
                            nc.scalar.activation(
                                e[:, 0:w], st[:, 0:w], Exp, scale=0.125)
                            if rel >= 0:
                                nc.vector.tensor_mul(
                                    e[:, 0:128], e[:, 0:128], trim[:])
                            nc.tensor.matmul(
                                cx[:, qoff:CH],
                                vv[:, i, h * 65:h * 65 + 65],
                                e[:, 0:w],
                                start=(i == 0), stop=(i == nst - 1))
                            if dbg and cj == 0 and h == 0:
                                ef = ats.tile([128, CH], f32, tag="dbgef")
                                nc.vector.tensor_copy(ef[:, 0:w], e[:, 0:w])
                                nc.sync.dma_start(
                                    out=dbg_e[i, :, 0:w], in_=ef[:, 0:w])
                        # softmax normalization: rows 0:64 / row 64.
                        # Reshape the den row to [128, CH//128] by DMA so the
                        # reciprocal uses all DVE lanes, then reshape back and
                        # broadcast across partitions with a K=1 matmul.
                        den_row = ats.tile([1, CH], f32, tag="den_row")
                        nc.scalar.copy(den_row[:], cx[64:65, :])
                        den_cols = ats.tile([128, CH // 128], f32, tag="den_cols")
                        nc.sync.dma_start(out=den_cols[:], in_=den_row[:])
                        rec_cols = ats.tile([128, CH // 128], bf16, tag="rec_cols")
                        nc.vector.reciprocal(rec_cols[:], den_cols[:])
                        rec_row = ats.tile([1, CH], bf16, tag="rec_row")
                        nc.sync.dma_start(out=rec_row[:], in_=rec_cols[:])
                        rbc = rbp.tile([64, CH], f32, tag="rbc")
                        nc.tensor.matmul(
                            rbc[:], ones[0:1, 0:64], rec_row[:],
                            start=True, stop=True)
                        cxb = ats.tile([64, CH], bf16, tag="cxb")
                        nc.vector.tensor_copy(cxb[:], cx[0:64, :])
                        nc.vector.tensor_mul(
                            cnp[pair][po:po + 64, :], cxb[:], rbc[:])
                        if dbg and cj == 0:
                            cnh = ats.tile([64, CH], f32, tag="dbgcnh")
                            nc.vector.tensor_copy(
                                cnh[:], cnp[pair][po:po + 64, :])
                            nc.sync.dma_start(out=dbg_cnh[h], in_=cnh[:])
                        if dbg and cj == 0 and h == 0:
                            cxd = ats.tile([65, CH], f32, tag="dbgcx")
                            nc.vector.tensor_copy(cxd[:], cx[:])
                            nc.sync.dma_start(out=dbg_cx[:], in_=cxd[:])
                            rbd = ats.tile([64, CH], f32, tag="dbgrb")
                            nc.vector.tensor_copy(rbd[:], rbc[:])
                            nc.sync.dma_start(out=dbg_rb[:], in_=rbd[:])
                            cnd = ats.tile([64, CH], f32, tag="dbgcn")
                            nc.vector.tensor_copy(
                                cnd[:], cnp[pair][po:po + 64, :])
                            nc.sync.dma_start(out=dbg_cn[:], in_=cnd[:])
                    # output projection for this chunk, all 4 heads
                    ew = min(512, d)
                    for tsub in range(CH // 128):
                        rows = slice(q0 + tsub * 128, q0 + tsub * 128 + 128)
                        for eh in range(d // ew):
                            es = slice(eh * ew, eh * ew + ew)
                            op = opp.tile([128, ew], f32, tag="op")
                            for p in range(2):
                                nc.tensor.matmul(
                                    op[:],
                                    cnp[p][:, tsub * 128:tsub * 128 + 128],
                                    wo[:, p, es],
                                    start=(p == 0), stop=(p == 1))
                            ob = ots.tile([128, ew], f32, tag="ob")
                            nc.vector.tensor_copy(ob[:], op[:])
                            nc.gpsimd.dma_start(out=outp[rows, es], in_=ob[:])
                            if dbg and cj == 0 and tsub == 0 and eh == 0:
                                nc.sync.dma_start(out=dbg_op[:], in_=ob[:])
    nc.finalize()
    return nc


def _prep_core_inputs(x, Wq, bq, Wk, bk, Wv, bv, Wo, bo, b, hg):
    sl = slice(hg * DHP, (hg + 1) * DHP)
    wvta = np.zeros((D, VW), np.float32)
    bva = np.zeros((1, VW), np.float32)
    wvt = Wv[sl, :].T.astype(np.float32)
    for h in range(HPC):
        wvta[:, h * 65:h * 65 + HD] = wvt[:, h * HD:(h + 1) * HD]
        bva[0, h * 65:h * 65 + HD] = bv[hg * DHP + h * HD: hg * DHP + (h + 1) * HD]
        bva[0, h * 65 + HD] = 1.0
    bqkc = np.stack([bq[sl].reshape(2, 128).T, bk[sl].reshape(2, 128).T],
                    axis=2).reshape(128, 4)
    # col order: [bq_pair0, bq_pair1, bk_pair0, bk_pair1]
    bqkc = np.concatenate([bq[sl].reshape(2, 128).T,
                           bk[sl].reshape(2, 128).T], axis=1)
    return {
        "xT": x[b].T.astype(BF),
        "wqT": Wq[sl, :].T.astype(BF),
        "wkT": Wk[sl, :].T.astype(BF),
        "wvTa": wvta.astype(BF),
        "woT": Wo[:, sl].T.reshape(2, 128, D).transpose(1, 0, 2).astype(BF),
        "bqkc": np.ascontiguousarray(bqkc, np.float32),
        "bvar": bva.astype(np.float32),
        "onesP": np.ones((128, 512), BF),
        "tri": np.triu(np.ones((128, 128), np.float32)).astype(BF),
    }


def kernel(x, Wq, bq, Wk, bk, Wv, bv, Wo, bo):
    from concourse.bass_utils import run_bass_kernel_spmd

    x = np.asarray(x, np.float32)
    args = [np.asarray(a, np.float32) for a in (Wq, bq, Wk, bk, Wv, bv, Wo, bo)]

    if "nc" not in _CACHE:
        _CACHE["nc"] = _build_nc(T, D)
    nc = _CACHE["nc"]

    in_maps = [
        _prep_core_inputs(x, *args, b=c // 4, hg=c % 4) for c in range(8)
    ]
    rr = run_bass_kernel_spmd(nc, in_maps, list(range(8)))
    _CACHE["last"] = rr
    res = rr.results

    bo = args[7]
    out = np.broadcast_to(bo, (T, D)).astype(np.float32)
    out = np.repeat(out[None], B, axis=0).copy()
    k = np.empty((B, H, T, HD), np.float32)
    v = np.empty((B, H, T, HD), np.float32)
    for c in range(8):
        b, hg = c // 4, c % 4
        hs = slice(hg * HPC, (hg + 1) * HPC)
        out[b] += res[c]["outp"]
        k[b, hs] = res[c]["kout"].reshape(HPC, HD, T).transpose(0, 2, 1)
        v[b, hs] = res[c]["vout"].reshape(T, HPC, HD).transpose(1, 0, 2)
    return (out, (k, v))


# revision 22
# speedup vs baseline: 1.0410x; 1.0410x over previous
"""Causal multi-head attention (B=2, T=2048, D=1024, H=16) on 8 TRN2 NeuronCores.

Sharding: core c handles batch b = c//4 and head-group hg = c%4 (4 heads each).
Per core, everything is computed in transposed-friendly layouts so no on-chip
transposes are needed:

  - qT, kT      [128=2 heads x 64, T]   (pair-stacked projections, bf16)
  - v'          [T-tiles, 4 heads x 65] (natural layout, 65th col = 1.0 so the
                                         PV matmul also produces the softmax
                                         denominator as psum row 64)
  - scores^T    [s=128, q<=512]  per s-tile, causal-restricted column range
  - softmax     exp on ACT (no max subtraction: |scores| is O(1) for this
                 problem), denominator via the ones-column, reciprocal on DVE,
                 broadcast across partitions with a K=1 PE matmul
  - out-proj    per-head K=64 matmuls accumulating all 4 heads into one PSUM
                tile; the final bias bo and the 4-way head-group reduction
                happen on the host (that reduction is the TP-unshard step).

Outputs per core: partial out [T, D] f32, kT [256, T] f32, v [T, 256] f32.
Host sums the 4 partials per batch, adds bo, and reassembles k/v to
[B, H, T, 64]. Returns (out, (k, v)) matching the reference pytree.
"""

import numpy as np
import ml_dtypes

B = 2
T = 2048
D = 1024
H = 16
HD = 64
HPC = 4            # heads per core
DHP = HPC * HD     # 256
VW = HPC * (HD + 1)  # 260, v' width incl. ones columns
CH = 512           # q-chunk width
BF = ml_dtypes.bfloat16

_CACHE = {}


def _build_nc(t, d, dbg=False):
    import concourse.bass as bass
    import concourse.tile as tile
    from concourse import bacc, mybir

    bf16 = mybir.dt.bfloat16
    f32 = mybir.dt.float32
    Exp = mybir.ActivationFunctionType.Exp

    ct = d // 128      # contraction tiles
    nt = t // 128      # 128-row t-tiles
    nch = t // CH      # q-chunks
    spc = CH // 128    # s-tiles per chunk (4)

    nc = bacc.Bacc()

    def act_recip(out_ap, in_ap):
        eng = nc.scalar
        ins = [eng.lower_ap(in_ap)]
        for val in (0.0, 1.0, 0.0):  # bias, scale, alpha
            ins.append(mybir.ImmediateValue(dtype=f32, value=val))
        return eng.add_instruction(mybir.InstActivation(
            name=nc.get_next_instruction_name(),
            func=mybir.ActivationFunctionType.Reciprocal,
            ins=ins, outs=[eng.lower_ap(out_ap)]))
    xT = nc.declare_dram_parameter("xT", [d, t], bf16, isOutput=False)
    wqT = nc.declare_dram_parameter("wqT", [d, DHP], bf16, isOutput=False)
    wkT = nc.declare_dram_parameter("wkT", [d, DHP], bf16, isOutput=False)
    wvTa = nc.declare_dram_parameter("wvTa", [d, VW], bf16, isOutput=False)
    woT = nc.declare_dram_parameter("woT", [128, 2, d], bf16, isOutput=False)
    bqkc = nc.declare_dram_parameter("bqkc", [128, 4], f32, isOutput=False)
    bvar = nc.declare_dram_parameter("bvar", [1, VW], f32, isOutput=False)
    onesP = nc.declare_dram_parameter("onesP", [128, 512], bf16, isOutput=False)
    tri = nc.declare_dram_parameter("tri", [128, 128], bf16, isOutput=False)

    outp = nc.declare_dram_parameter("outp", [t, d], f32, isOutput=True)
    kout = nc.declare_dram_parameter("kout", [DHP, t], f32, isOutput=True)
    vout = nc.declare_dram_parameter("vout", [t, DHP], f32, isOutput=True)
    if dbg:
        dbg_e = nc.dram_tensor("dbg_e", [CH // 128, 128, CH], f32,
                               kind="ExternalOutput")
        dbg_cx = nc.dram_tensor("dbg_cx", [65, CH], f32, kind="ExternalOutput")
        dbg_rb = nc.dram_tensor("dbg_rb", [64, CH], f32, kind="ExternalOutput")
        dbg_cn = nc.dram_tensor("dbg_cn", [64, CH], f32, kind="ExternalOutput")
        dbg_cnh = nc.dram_tensor("dbg_cnh", [HPC, 64, CH], f32,
                                 kind="ExternalOutput")
        dbg_op = nc.dram_tensor("dbg_op", [128, min(512, d)], f32,
                                kind="ExternalOutput")

    with tile.TileContext(nc) as tc, \
         nc.allow_low_precision(reason="bf16 compute pipeline"):
        with tc.tile_pool(name="persist", bufs=1) as pp:
            xt = pp.tile([128, ct, t], bf16)
            wq = pp.tile([128, ct, DHP], bf16)
            wk = pp.tile([128, ct, DHP], bf16)
            wv = pp.tile([128, ct, VW], bf16)
            wo = pp.tile([128, 2, d], bf16)
            bqk = pp.tile([128, 4], f32)
            ones = pp.tile([128, 512], bf16)
            trim = pp.tile([128, 128], bf16)
            qT = [pp.tile([128, t], bf16, name=f"qT{p}", tag=f"qT{p}") for p in range(2)]
            kT = [pp.tile([128, t], bf16, name=f"kT{p}", tag=f"kT{p}") for p in range(2)]
            vv = pp.tile([128, nt, VW], bf16)

            xTr = xT.rearrange("(a p) t -> p a t", p=128)
            nc.sync.dma_start(out=wq[:], in_=wqT.rearrange("(a p) m -> p a m", p=128))
            nc.scalar.dma_start(out=wk[:], in_=wkT.rearrange("(a p) m -> p a m", p=128))
            for a in range(ct):
                eng = nc.sync if a % 2 == 0 else nc.scalar
                eng.dma_start(out=xt[:, a, :], in_=xTr[:, a, :])
            nc.scalar.dma_start(out=wv[:], in_=wvTa.rearrange("(a p) m -> p a m", p=128))
            nc.sync.dma_start(out=wo[:], in_=woT[:])
            nc.sync.dma_start(out=bqk[:], in_=bqkc[:])
            # bias broadcast across partitions for the v evacuation
            bva_bc = pp.tile([128, VW], f32)
            _bap = bvar[:, :]
            nc.sync.dma_start(
                out=bva_bc[:],
                in_=bass.AP(tensor=_bap.tensor, offset=_bap.offset,
                            ap=[[0, 128]] + list(_bap.ap[1:])))
            nc.sync.dma_start(out=ones[:], in_=onesP[:])
            nc.sync.dma_start(out=trim[:], in_=tri[:])

            # ---- one scope: projections + attention interleave freely ----
            with tc.tile_pool(name="pj_ps", bufs=2, space="PSUM") as pjp, \
                 tc.tile_pool(name="pj_sb", bufs=3) as pjs, \
                 tc.tile_pool(name="st_ps", bufs=2, space="PSUM") as stp, \
                 tc.tile_pool(name="cx_ps", bufs=2, space="PSUM") as cxp, \
                 tc.tile_pool(name="rb_ps", bufs=1, space="PSUM") as rbp, \
                 tc.tile_pool(name="op_ps", bufs=1, space="PSUM") as opp, \
                 tc.tile_pool(name="at_sb", bufs=4) as ats, \
                 tc.tile_pool(name="cx_sb", bufs=8) as cxs, \
                 tc.tile_pool(name="ot_sb", bufs=4) as ots:
                for pair in range(2):
                    for tj in range(t // 512):
                        ts = slice(tj * 512, tj * 512 + 512)
                        ms = slice(pair * 128, pair * 128 + 128)
                        for bi, (name, w, dst) in enumerate(
                            (("q", wq, qT), ("k", wk, kT)),
                        ):
                            ps = pjp.tile([128, 512], f32, tag="pj")
                            for a in range(ct):
                                nc.tensor.matmul(
                                    ps[:], w[:, a, ms], xt[:, a, ts],
                                    start=(a == 0), stop=(a == ct - 1))
                            nc.vector.tensor_scalar_add(
                                dst[pair][:, ts], ps[:],
                                bqk[:, 2 * bi + pair: 2 * bi + pair + 1])
                            if name == "k":
                                kf = pjs.tile([128, 512], f32, tag="kf")
                                nc.vector.tensor_scalar_add(
                                    kf[:], ps[:],
                                    bqk[:, 2 * bi + pair: 2 * bi + pair + 1])
                                nc.gpsimd.dma_start(
                                    out=kout[ms, ts], in_=kf[:])
                for tj in range(nt):
                    ts = slice(tj * 128, tj * 128 + 128)
                    ps = pjp.tile([128, VW], f32, tag="pj")
                    for a in range(ct):
                        nc.tensor.matmul(
                            ps[:], xt[:, a, ts], wv[:, a, :],
                            start=(a == 0), stop=(a == ct - 1))
                    nc.vector.scalar_tensor_tensor(
                        out=vv[:, tj, :], in0=ps[:], scalar=1.0,
                        in1=bva_bc[:], op0=mybir.AluOpType.mult,
                        op1=mybir.AluOpType.add)
                    vf = pjs.tile([128, DHP], f32, tag="vf")
                    pv4 = ps.rearrange("p (h c) -> p h c", h=HPC)
                    bv4 = bva_bc.rearrange("p (h c) -> p h c", h=HPC)
                    nc.vector.scalar_tensor_tensor(
                        out=vf.rearrange("p (h c) -> p h c", h=HPC),
                        in0=pv4[:, :, 0:HD], scalar=1.0,
                        in1=bv4[:, :, 0:HD], op0=mybir.AluOpType.mult,
                        op1=mybir.AluOpType.add)
                    nc.gpsimd.dma_start(out=vout[ts, :], in_=vf[:])

                # ---- attention + output projection, per q-chunk ----
                for cj in range(nch):
                    q0 = cj * CH
                    cnp = [cxs.tile([128, CH], bf16, name=f"cnp{cj}_{p}",
                                    tag=f"cnp{p}") for p in range(2)]
                    for h in range(HPC):
                        pair, po = h // 2, (h % 2) * 64
                        qTh = qT[pair][po:po + 64, :]
                        kTh = kT[pair][po:po + 64, :]
                        nst = spc * (cj + 1)
                        cx = cxp.tile([65, CH], f32, tag="cx")
                        for i in range(nst):
                            rel = i - spc * cj
                            qoff = max(rel, 0) * 128
                            w = CH - qoff
                            st = stp.tile([128, CH], f32, tag="st")
                            nc.tensor.matmul(
                                st[:, 0:w],
                                kTh[:, i * 128:(i + 1) * 128],
                                qTh[:, q0 + qoff:q0 + CH],
                                start=True, stop=True)
                            e = ats.tile([128, CH], bf16, tag="e")
                            nc.scalar.activation(
                                e[:, 0:w], st[:, 0:w], Exp, scale=0.125)
                            if rel >= 0:
                                nc.vector.tensor_mul(
                                    e[:, 0:128], e[:, 0:128], trim[:])
                            nc.tensor.matmul(
                                cx[:, qoff:CH],
                                vv[:, i, h * 65:h * 65 + 65],
                                e[:, 0:w],
                                start=(i == 0), stop=(i == nst - 1))
                            if dbg and cj == 0 and h == 0:
                                ef = ats.tile([128, CH], f32, tag="dbgef")
                                nc.vector.tensor_copy(ef[:, 0:w], e[:, 0:w])
                                nc.sync.dma_start(
                                    out=dbg_e[i, :, 0:w], in_=ef[:, 0:w])
                        # softmax normalization: rows 0:64 / row 64.
                        # Reshape the den row to [128, CH//128] by DMA so the
                        # reciprocal uses all DVE lanes, then reshape back and
                        # broadcast across partitions with a K=1 matmul.
                        den_row = ats.tile([1, CH], f32, tag="den_row")
                        nc.vector.tensor_copy(den_row[:], cx[64:65, :])
                        den_cols = ats.tile([128, CH // 128], f32, tag="den_cols")
                        nc.sync.dma_start(out=den_cols[:], in_=den_row[:])
                        rec_cols = ats.tile([128, CH // 128], bf16, tag="rec_cols")
                        nc.vector.reciprocal(rec_cols[:], den_cols[:])
                        rec_row = ats.tile([1, CH], bf16, tag="rec_row")
                        nc.sync.dma_start(out=rec_row[:], in_=rec_cols[:])
                        rbc = rbp.tile([64, CH], f32, tag="rbc")
                        nc.tensor.matmul(
                            rbc[:], ones[0:1, 0:64], rec_row[:],
                            start=True, stop=True)
                        cxb = ats.tile([64, CH], bf16, tag="cxb")
                        nc.vector.tensor_copy(cxb[:], cx[0:64, :])
                        nc.vector.tensor_mul(
                            cnp[pair][po:po + 64, :], cxb[:], rbc[:])
                        if dbg and cj == 0:
                            cnh = ats.tile([64, CH], f32, tag="dbgcnh")
                            nc.vector.tensor_copy(
                                cnh[:], cnp[pair][po:po + 64, :])
                            nc.sync.dma_start(out=dbg_cnh[h], in_=cnh[:])
                        if dbg and cj == 0 and h == 0:
                            cxd = ats.tile([65, CH], f32, tag="dbgcx")
                            nc.vector.tensor_copy(cxd[:], cx[:])
                            nc.sync.dma_start(out=dbg_cx[:], in_=cxd[:])
                            rbd = ats.tile([64, CH], f32, tag="dbgrb")
                            nc.vector.tensor_copy(rbd[:], rbc[:])
                            nc.sync.dma_start(out=dbg_rb[:], in_=rbd[:])
                            cnd = ats.tile([64, CH], f32, tag="dbgcn")
                            nc.vector.tensor_copy(
                                cnd[:], cnp[pair][po:po + 64, :])
                            nc.sync.dma_start(out=dbg_cn[:], in_=cnd[:])
                    # output projection for this chunk, all 4 heads
                    ew = min(512, d)
                    for tsub in range(CH // 128):
                        rows = slice(q0 + tsub * 128, q0 + tsub * 128 + 128)
                        for eh in range(d // ew):
                            es = slice(eh * ew, eh * ew + ew)
                            op = opp.tile([128, ew], f32, tag="op")
                            for p in range(2):
                                nc.tensor.matmul(
                                    op[:],
                                    cnp[p][:, tsub * 128:tsub * 128 + 128],
                                    wo[:, p, es],
                                    start=(p == 0), stop=(p == 1))
                            ob = ots.tile([128, ew], f32, tag="ob")
                            nc.vector.tensor_copy(ob[:], op[:])
                            nc.gpsimd.dma_start(out=outp[rows, es], in_=ob[:])
                            if dbg and cj == 0 and tsub == 0 and eh == 0:
                                nc.sync.dma_start(out=dbg_op[:], in_=ob[:])
    nc.finalize()
    return nc


def _prep_core_inputs(x, Wq, bq, Wk, bk, Wv, bv, Wo, bo, b, hg):
    sl = slice(hg * DHP, (hg + 1) * DHP)
    wvta = np.zeros((D, VW), np.float32)
    bva = np.zeros((1, VW), np.float32)
    wvt = Wv[sl, :].T.astype(np.float32)
    for h in range(HPC):
        wvta[:, h * 65:h * 65 + HD] = wvt[:, h * HD:(h + 1) * HD]
        bva[0, h * 65:h * 65 + HD] = bv[hg * DHP + h * HD: hg * DHP + (h + 1) * HD]
        bva[0, h * 65 + HD] = 1.0
    bqkc = np.stack([bq[sl].reshape(2, 128).T, bk[sl].reshape(2, 128).T],
                    axis=2).reshape(128, 4)
    # col order: [bq_pair0, bq_pair1, bk_pair0, bk_pair1]
    bqkc = np.concatenate([bq[sl].reshape(2, 128).T,
                           bk[sl].reshape(2, 128).T], axis=1)
    return {
        "xT": x[b].T.astype(BF),
        "wqT": Wq[sl, :].T.astype(BF),
        "wkT": Wk[sl, :].T.astype(BF),
        "wvTa": wvta.astype(BF),
        "woT": Wo[:, sl].T.reshape(2, 128, D).transpose(1, 0, 2).astype(BF),
        "bqkc": np.ascontiguousarray(bqkc, np.float32),
        "bvar": bva.astype(np.float32),
        "onesP": np.ones((128, 512), BF),
        "tri": np.triu(np.ones((128, 128), np.float32)).astype(BF),
    }


def kernel(x, Wq, bq, Wk, bk, Wv, bv, Wo, bo):
    from concourse.bass_utils import run_bass_kernel_spmd

    x = np.asarray(x, np.float32)
    args = [np.asarray(a, np.float32) for a in (Wq, bq, Wk, bk, Wv, bv, Wo, bo)]

    if "nc" not in _CACHE:
        _CACHE["nc"] = _build_nc(T, D)
    nc = _CACHE["nc"]

    in_maps = [
        _prep_core_inputs(x, *args, b=c // 4, hg=c % 4) for c in range(8)
    ]
    rr = run_bass_kernel_spmd(nc, in_maps, list(range(8)))
    _CACHE["last"] = rr
    res = rr.results

    bo = args[7]
    out = np.broadcast_to(bo, (T, D)).astype(np.float32)
    out = np.repeat(out[None], B, axis=0).copy()
    k = np.empty((B, H, T, HD), np.float32)
    v = np.empty((B, H, T, HD), np.float32)
    for c in range(8):
        b, hg = c // 4, c % 4
        hs = slice(hg * HPC, (hg + 1) * HPC)
        out[b] += res[c]["outp"]
        k[b, hs] = res[c]["kout"].reshape(HPC, HD, T).transpose(0, 2, 1)
        v[b, hs] = res[c]["vout"].reshape(T, HPC, HD).transpose(1, 0, 2)
    return (out, (k, v))


# revision 23
# speedup vs baseline: 1.0639x; 1.0220x over previous
"""Causal multi-head attention (B=2, T=2048, D=1024, H=16) on 8 TRN2 NeuronCores.

Sharding: core c handles batch b = c//4 and head-group hg = c%4 (4 heads each).
Per core, everything is computed in transposed-friendly layouts so no on-chip
transposes are needed:

  - qT, kT      [128=2 heads x 64, T]   (pair-stacked projections, bf16)
  - v'          [T-tiles, 4 heads x 65] (natural layout, 65th col = 1.0 so the
                                         PV matmul also produces the softmax
                                         denominator as psum row 64)
  - scores^T    [s=128, q<=512]  per s-tile, causal-restricted column range
  - softmax     exp on ACT (no max subtraction: |scores| is O(1) for this
                 problem), denominator via the ones-column, reciprocal on DVE,
                 broadcast across partitions with a K=1 PE matmul
  - out-proj    per-head K=64 matmuls accumulating all 4 heads into one PSUM
                tile; the final bias bo and the 4-way head-group reduction
                happen on the host (that reduction is the TP-unshard step).

Outputs per core: partial out [T, D] f32, kT [256, T] f32, v [T, 256] f32.
Host sums the 4 partials per batch, adds bo, and reassembles k/v to
[B, H, T, 64]. Returns (out, (k, v)) matching the reference pytree.
"""

import numpy as np
import ml_dtypes

B = 2
T = 2048
D = 1024
H = 16
HD = 64
HPC = 4            # heads per core
DHP = HPC * HD     # 256
VW = HPC * (HD + 1)  # 260, v' width incl. ones columns
CH = 512           # q-chunk width
BF = ml_dtypes.bfloat16

_CACHE = {}


def _build_nc(t, d, dbg=False):
    import concourse.bass as bass
    import concourse.tile as tile
    from concourse import bacc, mybir

    bf16 = mybir.dt.bfloat16
    f32 = mybir.dt.float32
    Exp = mybir.ActivationFunctionType.Exp

    ct = d // 128      # contraction tiles
    nt = t // 128      # 128-row t-tiles
    nch = t // CH      # q-chunks
    spc = CH // 128    # s-tiles per chunk (4)

    nc = bacc.Bacc()

    def act_recip(out_ap, in_ap):
        eng = nc.scalar
        ins = [eng.lower_ap(in_ap)]
        for val in (0.0, 1.0, 0.0):  # bias, scale, alpha
            ins.append(mybir.ImmediateValue(dtype=f32, value=val))
        return eng.add_instruction(mybir.InstActivation(
            name=nc.get_next_instruction_name(),
            func=mybir.ActivationFunctionType.Reciprocal,
            ins=ins, outs=[eng.lower_ap(out_ap)]))
    xT = nc.declare_dram_parameter("xT", [d, t], bf16, isOutput=False)
    wqT = nc.declare_dram_parameter("wqT", [d, DHP], bf16, isOutput=False)
    wkT = nc.declare_dram_parameter("wkT", [d, DHP], bf16, isOutput=False)
    wvTa = nc.declare_dram_parameter("wvTa", [d, VW], bf16, isOutput=False)
    woT = nc.declare_dram_parameter("woT", [128, 2, d], bf16, isOutput=False)
    bqkc = nc.declare_dram_parameter("bqkc", [128, 4], f32, isOutput=False)
    bvar = nc.declare_dram_parameter("bvar", [1, VW], f32, isOutput=False)
    onesP = nc.declare_dram_parameter("onesP", [128, 512], bf16, isOutput=False)
    tri = nc.declare_dram_parameter("tri", [128, 128], bf16, isOutput=False)

    outp = nc.declare_dram_parameter("outp", [t, d], f32, isOutput=True)
    kout = nc.declare_dram_parameter("kout", [DHP, t], f32, isOutput=True)
    vout = nc.declare_dram_parameter("vout", [t, DHP], f32, isOutput=True)
    if dbg:
        dbg_e = nc.dram_tensor("dbg_e", [CH // 128, 128, CH], f32,
                               kind="ExternalOutput")
        dbg_cx = nc.dram_tensor("dbg_cx", [65, CH], f32, kind="ExternalOutput")
        dbg_rb = nc.dram_tensor("dbg_rb", [64, CH], f32, kind="ExternalOutput")
        dbg_cn = nc.dram_tensor("dbg_cn", [64, CH], f32, kind="ExternalOutput")
        dbg_cnh = nc.dram_tensor("dbg_cnh", [HPC, 64, CH], f32,
                                 kind="ExternalOutput")
        dbg_op = nc.dram_tensor("dbg_op", [128, min(512, d)], f32,
                                kind="ExternalOutput")

    with tile.TileContext(nc) as tc, \
         nc.allow_low_precision(reason="bf16 compute pipeline"):
        with tc.tile_pool(name="persist", bufs=1) as pp:
            xt = pp.tile([128, ct, t], bf16)
            wq = pp.tile([128, ct, DHP], bf16)
            wk = pp.tile([128, ct, DHP], bf16)
            wv = pp.tile([128, ct, VW], bf16)
            wo = pp.tile([128, 2, d], bf16)
            bqk = pp.tile([128, 4], f32)
            ones = pp.tile([128, 512], bf16)
            trim = pp.tile([128, 128], bf16)
            qT = [pp.tile([128, t], bf16, name=f"qT{p}", tag=f"qT{p}") for p in range(2)]
            kT = [pp.tile([128, t], bf16, name=f"kT{p}", tag=f"kT{p}") for p in range(2)]
            vv = pp.tile([128, nt, VW], bf16)

            xTr = xT.rearrange("(a p) t -> p a t", p=128)
            nc.sync.dma_start(out=wq[:], in_=wqT.rearrange("(a p) m -> p a m", p=128))
            nc.scalar.dma_start(out=wk[:], in_=wkT.rearrange("(a p) m -> p a m", p=128))
            for a in range(ct):
                eng = nc.sync if a % 2 == 0 else nc.scalar
                eng.dma_start(out=xt[:, a, :], in_=xTr[:, a, :])
            nc.scalar.dma_start(out=wv[:], in_=wvTa.rearrange("(a p) m -> p a m", p=128))
            nc.sync.dma_start(out=wo[:], in_=woT[:])
            nc.sync.dma_start(out=bqk[:], in_=bqkc[:])
            # bias broadcast across partitions for the v evacuation
            bva_bc = pp.tile([128, VW], f32)
            _bap = bvar[:, :]
            nc.sync.dma_start(
                out=bva_bc[:],
                in_=bass.AP(tensor=_bap.tensor, offset=_bap.offset,
                            ap=[[0, 128]] + list(_bap.ap[1:])))
            nc.sync.dma_start(out=ones[:], in_=onesP[:])
            nc.sync.dma_start(out=trim[:], in_=tri[:])

            # ---- one scope: projections + attention interleave freely ----
            with tc.tile_pool(name="pj_sb", bufs=3) as pjs, \
                 tc.tile_pool(name="st_ps", bufs=3, space="PSUM") as stp, \
                 tc.tile_pool(name="cx_ps", bufs=2, space="PSUM") as cxp, \
                 tc.tile_pool(name="rb_ps", bufs=1, space="PSUM") as rbp, \
                 tc.tile_pool(name="op_ps", bufs=2, space="PSUM") as opp, \
                 tc.tile_pool(name="at_sb", bufs=4) as ats, \
                 tc.tile_pool(name="cx_sb", bufs=8) as cxs, \
                 tc.tile_pool(name="ot_sb", bufs=4) as ots:
                for pair in range(2):
                    for tj in range(t // 512):
                        ts = slice(tj * 512, tj * 512 + 512)
                        ms = slice(pair * 128, pair * 128 + 128)
                        for bi, (name, w, dst) in enumerate(
                            (("q", wq, qT), ("k", wk, kT)),
                        ):
                            ps = stp.tile([128, 512], f32, tag="st")
                            for a in range(ct):
                                nc.tensor.matmul(
                                    ps[:], w[:, a, ms], xt[:, a, ts],
                                    start=(a == 0), stop=(a == ct - 1))
                            nc.vector.tensor_scalar_add(
                                dst[pair][:, ts], ps[:],
                                bqk[:, 2 * bi + pair: 2 * bi + pair + 1])
                            if name == "k":
                                kf = pjs.tile([128, 512], f32, tag="kf")
                                nc.vector.tensor_scalar_add(
                                    kf[:], ps[:],
                                    bqk[:, 2 * bi + pair: 2 * bi + pair + 1])
                                nc.gpsimd.dma_start(
                                    out=kout[ms, ts], in_=kf[:])
                for tj in range(nt):
                    ts = slice(tj * 128, tj * 128 + 128)
                    ps = stp.tile([128, VW], f32, tag="st")
                    for a in range(ct):
                        nc.tensor.matmul(
                            ps[:], xt[:, a, ts], wv[:, a, :],
                            start=(a == 0), stop=(a == ct - 1))
                    nc.vector.scalar_tensor_tensor(
                        out=vv[:, tj, :], in0=ps[:], scalar=1.0,
                        in1=bva_bc[:], op0=mybir.AluOpType.mult,
                        op1=mybir.AluOpType.add)
                    vf = pjs.tile([128, DHP], f32, tag="vf")
                    pv4 = ps.rearrange("p (h c) -> p h c", h=HPC)
                    bv4 = bva_bc.rearrange("p (h c) -> p h c", h=HPC)
                    nc.vector.scalar_tensor_tensor(
                        out=vf.rearrange("p (h c) -> p h c", h=HPC),
                        in0=pv4[:, :, 0:HD], scalar=1.0,
                        in1=bv4[:, :, 0:HD], op0=mybir.AluOpType.mult,
                        op1=mybir.AluOpType.add)
                    nc.gpsimd.dma_start(out=vout[ts, :], in_=vf[:])

                # ---- attention + output projection, per q-chunk ----
                for cj in reversed(range(nch)):
                    q0 = cj * CH
                    cnp = [cxs.tile([128, CH], bf16, name=f"cnp{cj}_{p}",
                                    tag=f"cnp{p}") for p in range(2)]
                    for h in range(HPC):
                        pair, po = h // 2, (h % 2) * 64
                        qTh = qT[pair][po:po + 64, :]
                        kTh = kT[pair][po:po + 64, :]
                        nst = spc * (cj + 1)
                        cx = cxp.tile([65, CH], f32, tag="cx")
                        for i in range(nst):
                            rel = i - spc * cj
                            qoff = max(rel, 0) * 128
                            w = CH - qoff
                            st = stp.tile([128, CH], f32, tag="st")
                            nc.tensor.matmul(
                                st[:, 0:w],
                                kTh[:, i * 128:(i + 1) * 128],
                                qTh[:, q0 + qoff:q0 + CH],
                                start=True, stop=True)
                            e = ats.tile([128, CH], bf16, tag="e")
                            nc.scalar.activation(
                                e[:, 0:w], st[:, 0:w], Exp, scale=0.125)
                            if rel >= 0:
                                nc.vector.tensor_mul(
                                    e[:, 0:128], e[:, 0:128], trim[:])
                            nc.tensor.matmul(
                                cx[:, qoff:CH],
                                vv[:, i, h * 65:h * 65 + 65],
                                e[:, 0:w],
                                start=(i == 0), stop=(i == nst - 1))
                            if dbg and cj == 0 and h == 0:
                                ef = ats.tile([128, CH], f32, tag="dbgef")
                                nc.vector.tensor_copy(ef[:, 0:w], e[:, 0:w])
                                nc.sync.dma_start(
                                    out=dbg_e[i, :, 0:w], in_=ef[:, 0:w])
                        # softmax normalization: rows 0:64 / row 64.
                        # Reshape the den row to [128, CH//128] by DMA so the
                        # reciprocal uses all DVE lanes, then reshape back and
                        # broadcast across partitions with a K=1 matmul.
                        den_row = ats.tile([1, CH], f32, tag="den_row")
                        nc.vector.tensor_copy(den_row[:], cx[64:65, :])
                        den_cols = ats.tile([128, CH // 128], f32, tag="den_cols")
                        nc.sync.dma_start(out=den_cols[:], in_=den_row[:])
                        rec_cols = ats.tile([128, CH // 128], bf16, tag="rec_cols")
                        nc.vector.reciprocal(rec_cols[:], den_cols[:])
                        rec_row = ats.tile([1, CH], bf16, tag="rec_row")
                        nc.sync.dma_start(out=rec_row[:], in_=rec_cols[:])
                        rbc = rbp.tile([64, CH], f32, tag="rbc")
                        nc.tensor.matmul(
                            rbc[:], ones[0:1, 0:64], rec_row[:],
                            start=True, stop=True)
                        cxb = ats.tile([64, CH], bf16, tag="cxb")
                        nc.vector.tensor_copy(cxb[:], cx[0:64, :])
                        nc.vector.tensor_mul(
                            cnp[pair][po:po + 64, :], cxb[:], rbc[:])
                        if dbg and cj == 0:
                            cnh = ats.tile([64, CH], f32, tag="dbgcnh")
                            nc.vector.tensor_copy(
                                cnh[:], cnp[pair][po:po + 64, :])
                            nc.sync.dma_start(out=dbg_cnh[h], in_=cnh[:])
                        if dbg and cj == 0 and h == 0:
                            cxd = ats.tile([65, CH], f32, tag="dbgcx")
                            nc.vector.tensor_copy(cxd[:], cx[:])
                            nc.sync.dma_start(out=dbg_cx[:], in_=cxd[:])
                            rbd = ats.tile([64, CH], f32, tag="dbgrb")
                            nc.vector.tensor_copy(rbd[:], rbc[:])
                            nc.sync.dma_start(out=dbg_rb[:], in_=rbd[:])
                            cnd = ats.tile([64, CH], f32, tag="dbgcn")
                            nc.vector.tensor_copy(
                                cnd[:], cnp[pair][po:po + 64, :])
                            nc.sync.dma_start(out=dbg_cn[:], in_=cnd[:])
                    # output projection for this chunk, all 4 heads
                    ew = min(512, d)
                    for tsub in range(CH // 128):
                        rows = slice(q0 + tsub * 128, q0 + tsub * 128 + 128)
                        for eh in range(d // ew):
                            es = slice(eh * ew, eh * ew + ew)
                            op = opp.tile([128, ew], f32, tag="op")
                            for p in range(2):
                                nc.tensor.matmul(
                                    op[:],
                                    cnp[p][:, tsub * 128:tsub * 128 + 128],
                                    wo[:, p, es],
                                    start=(p == 0), stop=(p == 1))
                            ob = ots.tile([128, ew], f32, tag="ob")
                            nc.vector.tensor_copy(ob[:], op[:])
                            nc.gpsimd.dma_start(out=outp[rows, es], in_=ob[:])
                            if dbg and cj == 0 and tsub == 0 and eh == 0:
                                nc.sync.dma_start(out=dbg_op[:], in_=ob[:])
    nc.finalize()
    return nc


def _prep_core_inputs(x, Wq, bq, Wk, bk, Wv, bv, Wo, bo, b, hg):
    sl = slice(hg * DHP, (hg + 1) * DHP)
    wvta = np.zeros((D, VW), np.float32)
    bva = np.zeros((1, VW), np.float32)
    wvt = Wv[sl, :].T.astype(np.float32)
    for h in range(HPC):
        wvta[:, h * 65:h * 65 + HD] = wvt[:, h * HD:(h + 1) * HD]
        bva[0, h * 65:h * 65 + HD] = bv[hg * DHP + h * HD: hg * DHP + (h + 1) * HD]
        bva[0, h * 65 + HD] = 1.0
    bqkc = np.stack([bq[sl].reshape(2, 128).T, bk[sl].reshape(2, 128).T],
                    axis=2).reshape(128, 4)
    # col order: [bq_pair0, bq_pair1, bk_pair0, bk_pair1]
    bqkc = np.concatenate([bq[sl].reshape(2, 128).T,
                           bk[sl].reshape(2, 128).T], axis=1)
    return {
        "xT": x[b].T.astype(BF),
        "wqT": Wq[sl, :].T.astype(BF),
        "wkT": Wk[sl, :].T.astype(BF),
        "wvTa": wvta.astype(BF),
        "woT": Wo[:, sl].T.reshape(2, 128, D).transpose(1, 0, 2).astype(BF),
        "bqkc": np.ascontiguousarray(bqkc, np.float32),
        "bvar": bva.astype(np.float32),
        "onesP": np.ones((128, 512), BF),
        "tri": np.triu(np.ones((128, 128), np.float32)).astype(BF),
    }


def kernel(x, Wq, bq, Wk, bk, Wv, bv, Wo, bo):
    from concourse.bass_utils import run_bass_kernel_spmd

    x = np.asarray(x, np.float32)
    args = [np.asarray(a, np.float32) for a in (Wq, bq, Wk, bk, Wv, bv, Wo, bo)]

    if "nc" not in _CACHE:
        _CACHE["nc"] = _build_nc(T, D)
    nc = _CACHE["nc"]

    in_maps = [
        _prep_core_inputs(x, *args, b=c // 4, hg=c % 4) for c in range(8)
    ]
    rr = run_bass_kernel_spmd(nc, in_maps, list(range(8)))
    _CACHE["last"] = rr
    res = rr.results

    bo = args[7]
    out = np.broadcast_to(bo, (T, D)).astype(np.float32)
    out = np.repeat(out[None], B, axis=0).copy()
    k = np.empty((B, H, T, HD), np.float32)
    v = np.empty((B, H, T, HD), np.float32)
    for c in range(8):
        b, hg = c // 4, c % 4
        hs = slice(hg * HPC, (hg + 1) * HPC)
        out[b] += res[c]["outp"]
        k[b, hs] = res[c]["kout"].reshape(HPC, HD, T).transpose(0, 2, 1)
        v[b, hs] = res[c]["vout"].reshape(T, HPC, HD).transpose(1, 0, 2)
    return (out, (k, v))
